# revision 2
# baseline (speedup 1.0000x reference)
"""AdaptiveRankLinear on Trainium2, 8-core data-parallel Bass/Tile kernel.

Computes  y = x + gamma * (((rmsnorm(x) * norm_weight) @ U) * (S*keep)) @ V
with keep = adaptive-rank mask from the singular-value energy of S.

Sharding: x is flattened to [8192, 4096] tokens and split into 8 shards of
1024 tokens (one per NeuronCore); U/S/V/norm_weight/gamma are tiny and
replicated (folded host-side into two small matrices).

v7: startup: weights (eye/u/v) issued on the ACT HWDGE queue concurrently
with x tiles on the Sync queue, x0 in quarters so transposes start early.
xT evacs all on DVE as uint32-bitcast copies (half the element count).
delta PSUM widened to [128,1024] (2 banks) x2 bufs; y done in 4 chunks of
1024 cols: 3 via DVE scalar_tensor_tensor, 1 via ACT copy-scale + DVE add.
ACT only does: square (sumsq accum), sqrt, one y copy-scale per tile.

Per-core device pipeline (per 128-token tile):
  DMA x(bf16) -> ACT square (accum -> sumsq) -> sqrt -> DVE recip -> rstd
  PE transpose x_j.T (bf16 psum, 8 blocks/bank) -> DVE uint32 evac
  PE hT += U2_j.T @ xT_j (f32 psum, 4 col strips) -> DVE evac bf16
  [one tile later] PE delta chunk = hT.T @ V2 (f32 psum, 2 MMs/chunk)
  DVE/ACT y = delta*rstd + x -> gpsimd DMA out (bf16) per 2048-col half.
"""
import ml_dtypes
import numpy as np

import concourse.bass as bass
import concourse.tile as tile
from concourse import mybir
from concourse.bass_utils import run_bass_kernel_spmd
from concourse.vector_clock import ScopedClock

# ----------------------------------------------------------------------------
# Workaround: this container's walrus accepts at most ONE sync wait per
# instruction, while Tile's sem-assigner can attach several.  Split extras
# into engine-local no-ops placed immediately before the over-waited
# instruction; same for the kernel-tail drain.
# ----------------------------------------------------------------------------
_MAXW = 1


def _split_bb_waits(nc, bb):
    insts = list(bb.instructions)
    out = []
    changed = False
    for inst in insts:
        si = inst.sync_info
        if si is not None and len(si.on_wait) > _MAXW:
            changed = True
            waits = list(si.on_wait)
            extra, keep = waits[:-_MAXW], waits[-_MAXW:]
            for k, w in enumerate(extra):
                nop = mybir.InstNoOp(name=f"{inst.name}_wsplit{k}", ins=[],
                                     outs=[])
                nop.engine = inst.engine
                nop.sync_info = mybir.SyncInfo(on_wait=[w], on_update=[])
                nc.register_instruction(nop, overwrite=True)
                out.append(nop)
            inst.sync_info = mybir.SyncInfo(on_wait=keep,
                                            on_update=list(si.on_update))
        out.append(inst)
    if changed:
        bb.instructions = out


def _patched_drain_and_barrier(self, tick_clock, wait_clock):
    for f in self.nc.m.functions:
        for bb in f.blocks:
            _split_bb_waits(self.nc, bb)

    drain_inst = self.nc.sync.drain()
    wait_clock.add_sem_waits(
        drain_inst.ins, ScopedClock({None: tick_clock.global_clock})
    )
    si = drain_inst.ins.sync_info
    if si is not None and len(si.on_wait) > _MAXW:
        waits = list(si.on_wait)
        drain_inst.ins.sync_info = mybir.SyncInfo(
            on_wait=waits[:_MAXW], on_update=list(si.on_update)
        )
        rest = waits[_MAXW:]
        for i in range(0, len(rest), _MAXW):
            nop = self.nc.sync.nop(nofuse=True, hint="drain_wait_spill")
            nop.ins.sync_info = mybir.SyncInfo(
                on_wait=rest[i:i + _MAXW], on_update=[]
            )

    self.nc.all_engine_barrier()
    assert self.sems is not None
    popped = self.nc._tile_sem_poison_stack.pop()
    assert popped is self._sem_poison
    self.nc.clear_and_free_semaphores(list(self.sems.allocated().values()))
    self.nc.all_engine_barrier()


tile.TileContext._drain_and_barrier = _patched_drain_and_barrier

# ----------------------------------------------------------------------------
# Problem constants (hardcoded; kernel.py must be self-contained).
# ----------------------------------------------------------------------------
N_CORES = 8
B, T, D = 4, 2048, 4096
TOK = B * T              # 8192
R = 16
SHARD = TOK // N_CORES   # 1024
PT = 128                 # tokens per tile
NT = SHARD // PT         # 8
KB = D // 128            # 32 contraction blocks
EPS = 1e-6
ENERGY_THRESHOLD = 0.95
F32 = mybir.dt.float32
BF16 = mybir.dt.bfloat16
U32 = mybir.dt.uint32
NP_BF16 = ml_dtypes.bfloat16
AF = mybir.ActivationFunctionType
ALU = mybir.AluOpType

TG = 8                   # transpose blocks per PSUM group (bank = 1024 bf16)
NG = KB // TG            # 4 groups per tile
CW = 1024                # y/delta chunk width (two f32 PSUM banks)
NC_CH = D // CW          # 4 chunks per tile
ACT_Y = (2,)             # y-chunks routed via ACT copy-scale + DVE add
RP = 32                  # U ranks padded to one 32-col PE strip
NSTRIP = 4               # concurrent col-tiled U strips


def build_nc():
    nc = bass.Bass("TRN2", target_bir_lowering=False, debug=False,
                   num_devices=N_CORES)
    x = nc.declare_dram_parameter("x", [SHARD, D], BF16, isOutput=False)
    u = nc.declare_dram_parameter("u", [128, KB * RP], BF16, isOutput=False)
    v = nc.declare_dram_parameter("v", [128, D], BF16, isOutput=False)
    eye = nc.declare_dram_parameter("eye", [PT, PT], BF16, isOutput=False)
    out = nc.declare_dram_parameter("out", [SHARD, D], BF16, isOutput=True)

    with tile.TileContext(nc) as tc:
        with (
            tc.tile_pool(name="singles", bufs=1) as singles,
            tc.tile_pool(name="xin", bufs=6) as xin,
            tc.tile_pool(name="xtp", bufs=2) as xtp,
            tc.tile_pool(name="yout", bufs=3) as yout,
            tc.tile_pool(name="smalls", bufs=4) as smalls,
            tc.tile_pool(name="keeps", bufs=3) as keeps,
            tc.tile_pool(name="sqd", bufs=1) as sqd,
            tc.tile_pool(name="scratch", bufs=2) as scratch,
            tc.tile_pool(name="xt_ps", bufs=2, space="PSUM") as xt_ps,
            tc.tile_pool(name="h_ps", bufs=2, space="PSUM") as h_ps,
            tc.tile_pool(name="d_ps", bufs=2, space="PSUM") as d_ps,
        ):
            # weights go out on the ACT HWDGE queue so the Sync queue is
            # free for x tiles from the first instruction.
            eye_sb = singles.tile([PT, PT], BF16)
            nc.scalar.dma_start(out=eye_sb, in_=eye[:, :])
            u_sb = singles.tile([128, KB, RP], BF16)
            nc.scalar.dma_start(out=u_sb,
                                in_=u.rearrange("p (k r) -> p k r", r=RP))
            v_sb = singles.tile([128, D], BF16)
            nc.scalar.dma_start(out=v_sb, in_=v[:, :])
            eps_sb = singles.tile([128, 1], F32)
            nc.vector.memset(eps_sb, EPS)

            prev = None  # expand-state of tile i-1

            def emit_expand_chunks(st, count):
                """Emit `count` (V-matmul pair + y) chunks of a pending tile."""
                if st is None:
                    return
                if st["y_sb"] is None:
                    st["y_sb"] = yout.tile([PT, D], BF16, name="y_sb",
                                           tag="y_sb")
                y_sb, rstd, x_sb, t0 = (st["y_sb"], st["rstd"], st["x_sb"],
                                        st["t0"])
                for _ in range(count):
                    n = st["n"]
                    if n >= NC_CH:
                        return
                    st["n"] = n + 1
                    dps = d_ps.tile([PT, CW], F32, tag="d")
                    c0 = n * CW
                    nc.tensor.matmul(out=dps[:, 0:CW // 2], lhsT=st["hs_sb"],
                                     rhs=v_sb[:, c0:c0 + CW // 2],
                                     start=True, stop=True)
                    nc.tensor.matmul(out=dps[:, CW // 2:CW],
                                     lhsT=st["hs_sb"],
                                     rhs=v_sb[:, c0 + CW // 2:c0 + CW],
                                     start=True, stop=True)
                    ysl = y_sb[:, c0:c0 + CW]
                    xsl = x_sb[:, c0:c0 + CW]
                    if n in ACT_Y:
                        dsb = scratch.tile([PT, CW], BF16, tag="dsb")
                        nc.scalar.activation(out=dsb, in_=dps, func=AF.Copy,
                                             scale=rstd)
                        nc.vector.tensor_add(out=ysl, in0=dsb, in1=xsl)
                    else:
                        nc.vector.scalar_tensor_tensor(
                            out=ysl, in0=dps, scalar=rstd, in1=xsl,
                            op0=ALU.mult, op1=ALU.add)
                    if n == NC_CH // 2 - 1:
                        nc.gpsimd.dma_start(out=out[t0:t0 + PT, :D // 2],
                                            in_=y_sb[:, :D // 2])
                    elif n == NC_CH - 1:
                        nc.gpsimd.dma_start(out=out[t0:t0 + PT, D // 2:],
                                            in_=y_sb[:, D // 2:])

            for it in range(NT):
                t0 = it * PT
                x_sb = xin.tile([PT, D], BF16, tag="x_sb")
                if it == 0:
                    # quarter-DMAs so the first transposes start early
                    for qd in range(4):
                        nc.sync.dma_start(
                            out=x_sb[:, qd * (D // 4):(qd + 1) * (D // 4)],
                            in_=x[t0:t0 + PT,
                                  qd * (D // 4):(qd + 1) * (D // 4)])
                else:
                    nc.sync.dma_start(out=x_sb, in_=x[t0:t0 + PT, :])

                # RMS stats: sumsq -> sqrt -> reciprocal -> rstd (fp32)
                sumsq = smalls.tile([PT, 1], F32, tag="sumsq")
                sq_a = sqd.tile([PT, D], BF16, tag="sq_a")
                nc.scalar.activation(out=sq_a, in_=x_sb, func=AF.Square,
                                     accum_out=sumsq)
                std = smalls.tile([PT, 1], F32, tag="std")
                nc.scalar.activation(out=std, in_=sumsq, func=AF.Sqrt,
                                     bias=eps_sb, scale=1.0 / D)
                rstd = keeps.tile([PT, 1], F32, tag="rstd")
                nc.vector.reciprocal(out=rstd, in_=std)

                # PE transpose: xT_j = x_j.T (bf16 psum) -> SBUF (u32 copy)
                xt_sb = xtp.tile([128, KB * PT], BF16)
                h_psum = h_ps.tile([128, PT], F32, tag="h")
                for g in range(NG):
                    tp = xt_ps.tile([128, TG * PT], BF16, tag="tp")
                    for q in range(TG):
                        j = g * TG + q
                        nc.tensor.matmul(
                            out=tp[:, q * PT:(q + 1) * PT],
                            lhsT=x_sb[:, j * 128:(j + 1) * 128],
                            rhs=eye_sb, is_transpose=True,
                            start=True, stop=True)
                    dst = xt_sb[:, g * TG * PT:(g + 1) * TG * PT]
                    nc.vector.tensor_copy(out=dst.bitcast(U32),
                                          in_=tp.bitcast(U32))

                    # h strip (j%4) += U2pad_j.T @ xT_j — 4 col-tiled strips
                    # run concurrently in distinct 32-column PE groups.
                    for q in range(TG):
                        j = g * TG + q
                        c = j % NSTRIP
                        nc.tensor.matmul(
                            out=h_psum[32 * c:32 * (c + 1), :],
                            lhsT=u_sb[:, j, :],
                            rhs=xt_sb[:, j * PT:(j + 1) * PT],
                            start=(j // NSTRIP == 0),
                            stop=(j // NSTRIP == KB // NSTRIP - 1),
                            tile_position=(0, 32 * c),
                            skip_group_check=True)

                    # interleave the previous tile's V/y chunks between
                    # groups so PE never head-of-line blocks on the delta
                    # bank rotation.
                    emit_expand_chunks(prev, 1)

                hs_sb = keeps.tile([128, PT], BF16, tag="hs")
                nc.vector.tensor_copy(out=hs_sb, in_=h_psum)
                emit_expand_chunks(prev, NC_CH)  # flush any remainder
                prev = {"hs_sb": hs_sb, "rstd": rstd, "x_sb": x_sb,
                        "t0": t0, "y_sb": None, "n": 0}

            emit_expand_chunks(prev, NC_CH)
    return nc


def _rank_mask_np(S):
    s_abs = np.abs(S)
    cum = np.cumsum(s_abs) / max(float(s_abs.sum()), 1e-8)
    hit = cum >= ENERGY_THRESHOLD
    r = int(np.argmax(hit)) + 1 if hit.any() else S.shape[0]
    return (np.arange(S.shape[0]) < r).astype(S.dtype)


def make_in_maps(x, U, S, V, norm_weight, gamma):
    S = np.asarray(S, dtype=np.float32)
    keep = _rank_mask_np(S)
    U2 = (np.asarray(norm_weight, dtype=np.float32)[:, None]
          * np.asarray(U, dtype=np.float32)
          * (S * keep)[None, :]).astype(NP_BF16)
    U2p = np.zeros((D, RP), dtype=NP_BF16)
    U2p[:, :R] = U2
    U2p = np.ascontiguousarray(
        U2p.reshape(KB, 128, RP).transpose(1, 0, 2).reshape(128, KB * RP))
    V2 = (np.asarray(V, dtype=np.float32)
          * np.asarray(gamma, dtype=np.float32)[None, :]).astype(NP_BF16)
    V2r = np.zeros((128, D), dtype=NP_BF16)
    for c in range(NSTRIP):
        V2r[32 * c:32 * c + R, :] = V2
    eye = np.eye(PT, dtype=NP_BF16)
    xf = np.ascontiguousarray(
        np.asarray(x, dtype=np.float32).reshape(TOK, D)).astype(NP_BF16)
    shards = np.split(xf, N_CORES, axis=0)
    return [{"x": s, "u": U2p, "v": V2r, "eye": eye} for s in shards]


_CACHED_NC = None


def run(x, U, S, V, norm_weight, gamma, trace=False, **kw):
    global _CACHED_NC
    if _CACHED_NC is None:
        _CACHED_NC = build_nc()
    in_maps = make_in_maps(x, U, S, V, norm_weight, gamma)
    res = run_bass_kernel_spmd(_CACHED_NC, in_maps,
                               core_ids=list(range(N_CORES)), trace=trace,
                               **kw)
    outs = [np.asarray(res.results[i]["out"]) for i in range(N_CORES)]
    y = np.concatenate(outs, axis=0).reshape(B, T, D).astype(np.float32)
    return y, res


def kernel(x, U, S, V, norm_weight, gamma):
    y, _ = run(x, U, S, V, norm_weight, gamma, trace=False)
    return y


# revision 9
# speedup vs baseline: 1.0870x; 1.0870x over previous
"""AdaptiveRankLinear on Trainium2, 8-core data-parallel Bass/Tile kernel.

Computes  y = x + gamma * (((rmsnorm(x) * norm_weight) @ U) * (S*keep)) @ V
with keep = adaptive-rank mask from the singular-value energy of S.

Sharding: x is flattened to [8192, 4096] tokens and split into 8 shards of
1024 tokens (one per NeuronCore); U/S/V/norm_weight/gamma are tiny and
replicated (folded host-side into two small matrices).

v8: DMA order on one queue: eye, x0 quarters, u, v, x1.. (weights after x0
so the first tile completes ~9us earlier).  delta PSUM [128,1024] (2
banks) x2 bufs; y in 4 chunks of 1024 cols: 3 via DVE stt, 1 via ACT
copy-scale + 2 GpSimd 512-wide adds (all-SBUF, the idle engine).  hs
PSUM->SBUF copy on ACT.  ACT: square/sqrt/hs/1 y-copy; DVE: evacs, 3 stt,
recip.

Per-core device pipeline (per 128-token tile):
  DMA x(bf16) -> ACT square (accum -> sumsq) -> sqrt -> DVE recip -> rstd
  PE transpose x_j.T (bf16 psum, 8 blocks/bank) -> DVE uint32 evac
  PE hT += U2_j.T @ xT_j (f32 psum, 4 col strips) -> DVE evac bf16
  [one tile later] PE delta chunk = hT.T @ V2 (f32 psum, 2 MMs/chunk)
  DVE/ACT y = delta*rstd + x -> gpsimd DMA out (bf16) per 2048-col half.
"""
import ml_dtypes
import numpy as np

import concourse.bass as bass
import concourse.tile as tile
from concourse import mybir
from concourse.bass_utils import run_bass_kernel_spmd
from concourse.vector_clock import ScopedClock

# ----------------------------------------------------------------------------
# Workaround: this container's walrus accepts at most ONE sync wait per
# instruction, while Tile's sem-assigner can attach several.  Split extras
# into engine-local no-ops placed immediately before the over-waited
# instruction; same for the kernel-tail drain.
# ----------------------------------------------------------------------------
_MAXW = 1


def _split_bb_waits(nc, bb):
    insts = list(bb.instructions)
    out = []
    changed = False
    for inst in insts:
        si = inst.sync_info
        if si is not None and len(si.on_wait) > _MAXW:
            changed = True
            waits = list(si.on_wait)
            extra, keep = waits[:-_MAXW], waits[-_MAXW:]
            for k, w in enumerate(extra):
                nop = mybir.InstNoOp(name=f"{inst.name}_wsplit{k}", ins=[],
                                     outs=[])
                nop.engine = inst.engine
                nop.sync_info = mybir.SyncInfo(on_wait=[w], on_update=[])
                nc.register_instruction(nop, overwrite=True)
                out.append(nop)
            inst.sync_info = mybir.SyncInfo(on_wait=keep,
                                            on_update=list(si.on_update))
        out.append(inst)
    if changed:
        bb.instructions = out


def _patched_drain_and_barrier(self, tick_clock, wait_clock):
    for f in self.nc.m.functions:
        for bb in f.blocks:
            _split_bb_waits(self.nc, bb)

    drain_inst = self.nc.sync.drain()
    wait_clock.add_sem_waits(
        drain_inst.ins, ScopedClock({None: tick_clock.global_clock})
    )
    si = drain_inst.ins.sync_info
    if si is not None and len(si.on_wait) > _MAXW:
        waits = list(si.on_wait)
        drain_inst.ins.sync_info = mybir.SyncInfo(
            on_wait=waits[:_MAXW], on_update=list(si.on_update)
        )
        rest = waits[_MAXW:]
        for i in range(0, len(rest), _MAXW):
            nop = self.nc.sync.nop(nofuse=True, hint="drain_wait_spill")
            nop.ins.sync_info = mybir.SyncInfo(
                on_wait=rest[i:i + _MAXW], on_update=[]
            )

    self.nc.all_engine_barrier()
    assert self.sems is not None
    popped = self.nc._tile_sem_poison_stack.pop()
    assert popped is self._sem_poison
    self.nc.clear_and_free_semaphores(list(self.sems.allocated().values()))
    self.nc.all_engine_barrier()


tile.TileContext._drain_and_barrier = _patched_drain_and_barrier

# ----------------------------------------------------------------------------
# Problem constants (hardcoded; kernel.py must be self-contained).
# ----------------------------------------------------------------------------
N_CORES = 8
B, T, D = 4, 2048, 4096
TOK = B * T              # 8192
R = 16
SHARD = TOK // N_CORES   # 1024
PT = 128                 # tokens per tile
NT = SHARD // PT         # 8
KB = D // 128            # 32 contraction blocks
EPS = 1e-6
ENERGY_THRESHOLD = 0.95
F32 = mybir.dt.float32
BF16 = mybir.dt.bfloat16
U32 = mybir.dt.uint32
NP_BF16 = ml_dtypes.bfloat16
AF = mybir.ActivationFunctionType
ALU = mybir.AluOpType

TG = 8                   # transpose blocks per PSUM group (bank = 1024 bf16)
NG = KB // TG            # 4 groups per tile
CW = 1024                # y/delta chunk width (two f32 PSUM banks)
NC_CH = D // CW          # 4 chunks per tile
ACT_Y = (1,)             # y-chunks routed via ACT copy-scale + GpSimd adds
RP = 32                  # U ranks padded to one 32-col PE strip
NSTRIP = 4               # concurrent col-tiled U strips


def build_nc():
    nc = bass.Bass("TRN2", target_bir_lowering=False, debug=False,
                   num_devices=N_CORES)
    x = nc.declare_dram_parameter("x", [SHARD, D], BF16, isOutput=False)
    u = nc.declare_dram_parameter("u", [128, KB * RP], BF16, isOutput=False)
    v = nc.declare_dram_parameter("v", [128, D], BF16, isOutput=False)
    eye = nc.declare_dram_parameter("eye", [PT, PT], BF16, isOutput=False)
    out = nc.declare_dram_parameter("out", [SHARD, D], BF16, isOutput=True)

    with tile.TileContext(nc) as tc:
        with (
            tc.tile_pool(name="singles", bufs=1) as singles,
            tc.tile_pool(name="xin", bufs=6) as xin,
            tc.tile_pool(name="xtp", bufs=2) as xtp,
            tc.tile_pool(name="yout", bufs=3) as yout,
            tc.tile_pool(name="smalls", bufs=4) as smalls,
            tc.tile_pool(name="keeps", bufs=3) as keeps,
            tc.tile_pool(name="sqd", bufs=1) as sqd,
            tc.tile_pool(name="scratch", bufs=2) as scratch,
            tc.tile_pool(name="xt_ps", bufs=3, space="PSUM") as xt_ps,
            tc.tile_pool(name="h_ps", bufs=1, space="PSUM") as h_ps,
            tc.tile_pool(name="d_ps", bufs=2, space="PSUM") as d_ps,
        ):
            # eye goes first (transposes need it, and it is tiny); u/v are
            # issued inside the it==0 body AFTER the x0 quarters so the
            # first tile is not delayed behind 1.3MB of weights.
            eye_sb = singles.tile([PT, PT], BF16)
            nc.sync.dma_start(out=eye_sb, in_=eye[:, :])
            u_sb = singles.tile([128, KB, RP], BF16)
            v_sb = singles.tile([128, D], BF16)
            eps_sb = singles.tile([128, 1], F32)
            nc.vector.memset(eps_sb, EPS)

            prev = None  # expand-state of tile i-1

            def emit_expand_chunks(st, count):
                """Emit `count` (V-matmul pair + y) chunks of a pending tile."""
                if st is None:
                    return
                if st["y_sb"] is None:
                    st["y_sb"] = yout.tile([PT, D], BF16, name="y_sb",
                                           tag="y_sb")
                y_sb, rstd, x_sb, t0 = (st["y_sb"], st["rstd"], st["x_sb"],
                                        st["t0"])
                for _ in range(count):
                    n = st["n"]
                    if n >= NC_CH:
                        return
                    st["n"] = n + 1
                    dps = d_ps.tile([PT, CW], F32, tag="d")
                    c0 = n * CW
                    nc.tensor.matmul(out=dps[:, 0:CW // 2], lhsT=st["hs_sb"],
                                     rhs=v_sb[:, c0:c0 + CW // 2],
                                     start=True, stop=True)
                    nc.tensor.matmul(out=dps[:, CW // 2:CW],
                                     lhsT=st["hs_sb"],
                                     rhs=v_sb[:, c0 + CW // 2:c0 + CW],
                                     start=True, stop=True)
                    ysl = y_sb[:, c0:c0 + CW]
                    xsl = x_sb[:, c0:c0 + CW]
                    if n in ACT_Y:
                        dsb = scratch.tile([PT, CW], BF16, tag="dsb")
                        nc.scalar.activation(out=dsb, in_=dps, func=AF.Copy,
                                             scale=rstd)
                        # all-SBUF adds on the otherwise-idle GpSimd engine,
                        # split in two so each stays ~1us
                        hw = CW // 2
                        nc.gpsimd.tensor_add(out=ysl[:, :hw],
                                             in0=dsb[:, :hw],
                                             in1=xsl[:, :hw])
                        nc.gpsimd.tensor_add(out=ysl[:, hw:],
                                             in0=dsb[:, hw:],
                                             in1=xsl[:, hw:])
                    else:
                        nc.vector.scalar_tensor_tensor(
                            out=ysl, in0=dps, scalar=rstd, in1=xsl,
                            op0=ALU.mult, op1=ALU.add)
                    if n == NC_CH // 2 - 1:
                        nc.gpsimd.dma_start(out=out[t0:t0 + PT, :D // 2],
                                            in_=y_sb[:, :D // 2])
                    elif n == NC_CH - 1:
                        nc.gpsimd.dma_start(out=out[t0:t0 + PT, D // 2:],
                                            in_=y_sb[:, D // 2:])

            for it in range(NT):
                t0 = it * PT
                x_sb = xin.tile([PT, D], BF16, tag="x_sb")
                if it == 0:
                    # quarter-DMAs so the first transposes start early
                    for qd in range(4):
                        nc.sync.dma_start(
                            out=x_sb[:, qd * (D // 4):(qd + 1) * (D // 4)],
                            in_=x[t0:t0 + PT,
                                  qd * (D // 4):(qd + 1) * (D // 4)])
                    nc.sync.dma_start(
                        out=u_sb, in_=u.rearrange("p (k r) -> p k r", r=RP))
                    nc.sync.dma_start(out=v_sb, in_=v[:, :])
                else:
                    nc.sync.dma_start(out=x_sb, in_=x[t0:t0 + PT, :])

                # RMS stats: sumsq -> sqrt -> reciprocal -> rstd (fp32)
                sumsq = smalls.tile([PT, 1], F32, tag="sumsq")
                sq_a = sqd.tile([PT, D], BF16, tag="sq_a")
                nc.scalar.activation(out=sq_a, in_=x_sb, func=AF.Square,
                                     accum_out=sumsq)
                std = smalls.tile([PT, 1], F32, tag="std")
                nc.scalar.activation(out=std, in_=sumsq, func=AF.Sqrt,
                                     bias=eps_sb, scale=1.0 / D)
                rstd = keeps.tile([PT, 1], F32, tag="rstd")
                nc.vector.reciprocal(out=rstd, in_=std)

                # PE transpose: xT_j = x_j.T (bf16 psum) -> SBUF (u32 copy)
                xt_sb = xtp.tile([128, KB * PT], BF16)
                h_psum = h_ps.tile([128, PT], F32, tag="h")
                for g in range(NG):
                    tp = xt_ps.tile([128, TG * PT], BF16, tag="tp")
                    for q in range(TG):
                        j = g * TG + q
                        nc.tensor.matmul(
                            out=tp[:, q * PT:(q + 1) * PT],
                            lhsT=x_sb[:, j * 128:(j + 1) * 128],
                            rhs=eye_sb, is_transpose=True,
                            start=True, stop=True)
                    dst = xt_sb[:, g * TG * PT:(g + 1) * TG * PT]
                    nc.vector.tensor_copy(out=dst, in_=tp)

                    # h strip (j%4) += U2pad_j.T @ xT_j — 4 col-tiled strips
                    # run concurrently in distinct 32-column PE groups.
                    for q in range(TG):
                        j = g * TG + q
                        c = j % NSTRIP
                        nc.tensor.matmul(
                            out=h_psum[32 * c:32 * (c + 1), :],
                            lhsT=u_sb[:, j, :],
                            rhs=xt_sb[:, j * PT:(j + 1) * PT],
                            start=(j // NSTRIP == 0),
                            stop=(j // NSTRIP == KB // NSTRIP - 1),
                            tile_position=(0, 32 * c),
                            skip_group_check=True)

                    # interleave the previous tile's V/y chunks between
                    # groups so PE never head-of-line blocks on the delta
                    # bank rotation.
                    emit_expand_chunks(prev, 1)

                hs_sb = keeps.tile([128, PT], BF16, tag="hs")
                nc.scalar.copy(out=hs_sb, in_=h_psum)
                emit_expand_chunks(prev, NC_CH)  # flush any remainder
                prev = {"hs_sb": hs_sb, "rstd": rstd, "x_sb": x_sb,
                        "t0": t0, "y_sb": None, "n": 0}

            emit_expand_chunks(prev, NC_CH)
    return nc


def _rank_mask_np(S):
    s_abs = np.abs(S)
    cum = np.cumsum(s_abs) / max(float(s_abs.sum()), 1e-8)
    hit = cum >= ENERGY_THRESHOLD
    r = int(np.argmax(hit)) + 1 if hit.any() else S.shape[0]
    return (np.arange(S.shape[0]) < r).astype(S.dtype)


def make_in_maps(x, U, S, V, norm_weight, gamma):
    S = np.asarray(S, dtype=np.float32)
    keep = _rank_mask_np(S)
    U2 = (np.asarray(norm_weight, dtype=np.float32)[:, None]
          * np.asarray(U, dtype=np.float32)
          * (S * keep)[None, :]).astype(NP_BF16)
    U2p = np.zeros((D, RP), dtype=NP_BF16)
    U2p[:, :R] = U2
    U2p = np.ascontiguousarray(
        U2p.reshape(KB, 128, RP).transpose(1, 0, 2).reshape(128, KB * RP))
    V2 = (np.asarray(V, dtype=np.float32)
          * np.asarray(gamma, dtype=np.float32)[None, :]).astype(NP_BF16)
    V2r = np.zeros((128, D), dtype=NP_BF16)
    for c in range(NSTRIP):
        V2r[32 * c:32 * c + R, :] = V2
    eye = np.eye(PT, dtype=NP_BF16)
    xf = np.ascontiguousarray(
        np.asarray(x, dtype=np.float32).reshape(TOK, D)).astype(NP_BF16)
    shards = np.split(xf, N_CORES, axis=0)
    return [{"x": s, "u": U2p, "v": V2r, "eye": eye} for s in shards]


_CACHED_NC = None


def run(x, U, S, V, norm_weight, gamma, trace=False, **kw):
    global _CACHED_NC
    if _CACHED_NC is None:
        _CACHED_NC = build_nc()
    in_maps = make_in_maps(x, U, S, V, norm_weight, gamma)
    res = run_bass_kernel_spmd(_CACHED_NC, in_maps,
                               core_ids=list(range(N_CORES)), trace=trace,
                               **kw)
    outs = [np.asarray(res.results[i]["out"]) for i in range(N_CORES)]
    y = np.concatenate(outs, axis=0).reshape(B, T, D).astype(np.float32)
    return y, res


def kernel(x, U, S, V, norm_weight, gamma):
    y, _ = run(x, U, S, V, norm_weight, gamma, trace=False)
    return y


# revision 11
# speedup vs baseline: 1.1028x; 1.0146x over previous
"""AdaptiveRankLinear on Trainium2, 8-core data-parallel Bass/Tile kernel.

Computes  y = x + gamma * (((rmsnorm(x) * norm_weight) @ U) * (S*keep)) @ V
with keep = adaptive-rank mask from the singular-value energy of S.

Sharding: x is flattened to [8192, 4096] tokens and split into 8 shards of
1024 tokens (one per NeuronCore); U/S/V/norm_weight/gamma are tiny and
replicated (folded host-side into two small matrices).

v8: DMA order on one queue: eye, x0 quarters, u, v, x1.. (weights after x0
so the first tile completes ~9us earlier).  delta PSUM [128,1024] (2
banks) x2 bufs; y in 4 chunks of 1024 cols: 3 via DVE stt, 1 via ACT
copy-scale + 2 GpSimd 512-wide adds (all-SBUF, the idle engine).  hs
PSUM->SBUF copy on ACT.  ACT: square/sqrt/hs/1 y-copy; DVE: evacs, 3 stt,
recip.

Per-core device pipeline (per 128-token tile):
  DMA x(bf16) -> ACT square (accum -> sumsq) -> sqrt -> DVE recip -> rstd
  PE transpose x_j.T (bf16 psum, 8 blocks/bank) -> DVE uint32 evac
  PE hT += U2_j.T @ xT_j (f32 psum, 4 col strips) -> DVE evac bf16
  [one tile later] PE delta chunk = hT.T @ V2 (f32 psum, 2 MMs/chunk)
  DVE/ACT y = delta*rstd + x -> gpsimd DMA out (bf16) per 2048-col half.
"""
import ml_dtypes
import numpy as np

import concourse.bass as bass
import concourse.tile as tile
from concourse import mybir
from concourse.bass_utils import run_bass_kernel_spmd
from concourse.vector_clock import ScopedClock

# ----------------------------------------------------------------------------
# Workaround: this container's walrus accepts at most ONE sync wait per
# instruction, while Tile's sem-assigner can attach several.  Split extras
# into engine-local no-ops placed immediately before the over-waited
# instruction; same for the kernel-tail drain.
# ----------------------------------------------------------------------------
_MAXW = 1


def _split_bb_waits(nc, bb):
    insts = list(bb.instructions)
    out = []
    changed = False
    for inst in insts:
        si = inst.sync_info
        if si is not None and len(si.on_wait) > _MAXW:
            changed = True
            waits = list(si.on_wait)
            extra, keep = waits[:-_MAXW], waits[-_MAXW:]
            for k, w in enumerate(extra):
                nop = mybir.InstNoOp(name=f"{inst.name}_wsplit{k}", ins=[],
                                     outs=[])
                nop.engine = inst.engine
                nop.sync_info = mybir.SyncInfo(on_wait=[w], on_update=[])
                nc.register_instruction(nop, overwrite=True)
                out.append(nop)
            inst.sync_info = mybir.SyncInfo(on_wait=keep,
                                            on_update=list(si.on_update))
        out.append(inst)
    if changed:
        bb.instructions = out


def _patched_drain_and_barrier(self, tick_clock, wait_clock):
    for f in self.nc.m.functions:
        for bb in f.blocks:
            _split_bb_waits(self.nc, bb)

    drain_inst = self.nc.sync.drain()
    wait_clock.add_sem_waits(
        drain_inst.ins, ScopedClock({None: tick_clock.global_clock})
    )
    si = drain_inst.ins.sync_info
    if si is not None and len(si.on_wait) > _MAXW:
        waits = list(si.on_wait)
        drain_inst.ins.sync_info = mybir.SyncInfo(
            on_wait=waits[:_MAXW], on_update=list(si.on_update)
        )
        rest = waits[_MAXW:]
        for i in range(0, len(rest), _MAXW):
            nop = self.nc.sync.nop(nofuse=True, hint="drain_wait_spill")
            nop.ins.sync_info = mybir.SyncInfo(
                on_wait=rest[i:i + _MAXW], on_update=[]
            )

    self.nc.all_engine_barrier()
    assert self.sems is not None
    popped = self.nc._tile_sem_poison_stack.pop()
    assert popped is self._sem_poison
    self.nc.clear_and_free_semaphores(list(self.sems.allocated().values()))
    self.nc.all_engine_barrier()


tile.TileContext._drain_and_barrier = _patched_drain_and_barrier

# ----------------------------------------------------------------------------
# Problem constants (hardcoded; kernel.py must be self-contained).
# ----------------------------------------------------------------------------
N_CORES = 8
B, T, D = 4, 2048, 4096
TOK = B * T              # 8192
R = 16
SHARD = TOK // N_CORES   # 1024
PT = 128                 # tokens per tile
NT = SHARD // PT         # 8
KB = D // 128            # 32 contraction blocks
EPS = 1e-6
ENERGY_THRESHOLD = 0.95
F32 = mybir.dt.float32
BF16 = mybir.dt.bfloat16
U32 = mybir.dt.uint32
NP_BF16 = ml_dtypes.bfloat16
AF = mybir.ActivationFunctionType
ALU = mybir.AluOpType

TG = 8                   # transpose blocks per PSUM group (bank = 1024 bf16)
NG = KB // TG            # 4 groups per tile
CW = 1024                # y/delta chunk width (two f32 PSUM banks)
NC_CH = D // CW          # 4 chunks per tile
ACT_Y = (3,)             # y-chunks routed via ACT copy-scale + GpSimd adds
                         # (chunk 3: its ACT op lands before the next
                         # tile's square in the ACT queue, so the d_ps WAR
                         # chain never waits behind a 3.7us square)
RP = 32                  # U ranks padded to one 32-col PE strip
NSTRIP = 4               # concurrent col-tiled U strips


def build_nc():
    nc = bass.Bass("TRN2", target_bir_lowering=False, debug=False,
                   num_devices=N_CORES)
    x = nc.declare_dram_parameter("x", [SHARD, D], BF16, isOutput=False)
    u = nc.declare_dram_parameter("u", [128, KB * RP], BF16, isOutput=False)
    v = nc.declare_dram_parameter("v", [128, D], BF16, isOutput=False)
    eye = nc.declare_dram_parameter("eye", [PT, PT], BF16, isOutput=False)
    out = nc.declare_dram_parameter("out", [SHARD, D], BF16, isOutput=True)

    with tile.TileContext(nc) as tc:
        with (
            tc.tile_pool(name="singles", bufs=1) as singles,
            tc.tile_pool(name="xin", bufs=7) as xin,
            tc.tile_pool(name="xtp", bufs=2) as xtp,
            tc.tile_pool(name="yout", bufs=3) as yout,
            tc.tile_pool(name="smalls", bufs=4) as smalls,
            tc.tile_pool(name="keeps", bufs=3) as keeps,
            tc.tile_pool(name="sqd", bufs=1) as sqd,
            tc.tile_pool(name="scratch", bufs=2) as scratch,
            tc.tile_pool(name="xt_ps", bufs=3, space="PSUM") as xt_ps,
            tc.tile_pool(name="h_ps", bufs=1, space="PSUM") as h_ps,
            tc.tile_pool(name="d_ps", bufs=2, space="PSUM") as d_ps,
        ):
            # eye goes first (transposes need it, and it is tiny); u/v are
            # issued inside the it==0 body AFTER the x0 quarters so the
            # first tile is not delayed behind 1.3MB of weights.
            eye_sb = singles.tile([PT, PT], BF16)
            nc.sync.dma_start(out=eye_sb, in_=eye[:, :])
            u_sb = singles.tile([128, KB, RP], BF16)
            v_sb = singles.tile([128, D], BF16)
            eps_sb = singles.tile([128, 1], F32)
            nc.vector.memset(eps_sb, EPS)

            prev = None  # expand-state of tile i-1

            def emit_expand_chunks(st, count):
                """Emit `count` (V-matmul pair + y) chunks of a pending tile."""
                if st is None:
                    return
                if st["y_sb"] is None:
                    st["y_sb"] = yout.tile([PT, D], BF16, name="y_sb",
                                           tag="y_sb")
                y_sb, rstd, x_sb, t0 = (st["y_sb"], st["rstd"], st["x_sb"],
                                        st["t0"])
                for _ in range(count):
                    n = st["n"]
                    if n >= NC_CH:
                        return
                    st["n"] = n + 1
                    dps = d_ps.tile([PT, CW], F32, tag="d")
                    c0 = n * CW
                    nc.tensor.matmul(out=dps[:, 0:CW // 2], lhsT=st["hs_sb"],
                                     rhs=v_sb[:, c0:c0 + CW // 2],
                                     start=True, stop=True)
                    nc.tensor.matmul(out=dps[:, CW // 2:CW],
                                     lhsT=st["hs_sb"],
                                     rhs=v_sb[:, c0 + CW // 2:c0 + CW],
                                     start=True, stop=True)
                    ysl = y_sb[:, c0:c0 + CW]
                    xsl = x_sb[:, c0:c0 + CW]
                    if n in ACT_Y:
                        dsb = scratch.tile([PT, CW], BF16, tag="dsb")
                        nc.scalar.activation(out=dsb, in_=dps, func=AF.Copy,
                                             scale=rstd)
                        # all-SBUF adds on the otherwise-idle GpSimd engine,
                        # split in two so each stays ~1us
                        hw = CW // 2
                        nc.gpsimd.tensor_add(out=ysl[:, :hw],
                                             in0=dsb[:, :hw],
                                             in1=xsl[:, :hw])
                        nc.gpsimd.tensor_add(out=ysl[:, hw:],
                                             in0=dsb[:, hw:],
                                             in1=xsl[:, hw:])
                    else:
                        nc.vector.scalar_tensor_tensor(
                            out=ysl, in0=dps, scalar=rstd, in1=xsl,
                            op0=ALU.mult, op1=ALU.add)
                    if n == NC_CH // 2 - 1:
                        nc.gpsimd.dma_start(out=out[t0:t0 + PT, :D // 2],
                                            in_=y_sb[:, :D // 2])
                    elif n == NC_CH - 1:
                        nc.gpsimd.dma_start(out=out[t0:t0 + PT, D // 2:],
                                            in_=y_sb[:, D // 2:])

            for it in range(NT):
                t0 = it * PT
                x_sb = xin.tile([PT, D], BF16, tag="x_sb")
                if it == 0:
                    # quarter-DMAs so the first transposes start early
                    for qd in range(4):
                        nc.sync.dma_start(
                            out=x_sb[:, qd * (D // 4):(qd + 1) * (D // 4)],
                            in_=x[t0:t0 + PT,
                                  qd * (D // 4):(qd + 1) * (D // 4)])
                    nc.sync.dma_start(
                        out=u_sb, in_=u.rearrange("p (k r) -> p k r", r=RP))
                    nc.sync.dma_start(out=v_sb, in_=v[:, :])
                else:
                    nc.sync.dma_start(out=x_sb, in_=x[t0:t0 + PT, :])

                # RMS stats: sumsq -> sqrt -> reciprocal -> rstd (fp32)
                sumsq = smalls.tile([PT, 1], F32, tag="sumsq")
                sq_a = sqd.tile([PT, D], BF16, tag="sq_a")
                nc.scalar.activation(out=sq_a, in_=x_sb, func=AF.Square,
                                     accum_out=sumsq)
                std = smalls.tile([PT, 1], F32, tag="std")
                nc.scalar.activation(out=std, in_=sumsq, func=AF.Sqrt,
                                     bias=eps_sb, scale=1.0 / D)
                rstd = keeps.tile([PT, 1], F32, tag="rstd")
                nc.vector.reciprocal(out=rstd, in_=std)

                # PE transpose: xT_j = x_j.T (bf16 psum) -> SBUF (u32 copy)
                xt_sb = xtp.tile([128, KB * PT], BF16)
                h_psum = h_ps.tile([128, PT], F32, tag="h")
                for g in range(NG):
                    tp = xt_ps.tile([128, TG * PT], BF16, tag="tp")
                    for q in range(TG):
                        j = g * TG + q
                        nc.tensor.matmul(
                            out=tp[:, q * PT:(q + 1) * PT],
                            lhsT=x_sb[:, j * 128:(j + 1) * 128],
                            rhs=eye_sb, is_transpose=True,
                            start=True, stop=True)
                    dst = xt_sb[:, g * TG * PT:(g + 1) * TG * PT]
                    nc.vector.tensor_copy(out=dst, in_=tp)

                    # h strip (j%4) += U2pad_j.T @ xT_j — 4 col-tiled strips
                    # run concurrently in distinct 32-column PE groups.
                    for q in range(TG):
                        j = g * TG + q
                        c = j % NSTRIP
                        nc.tensor.matmul(
                            out=h_psum[32 * c:32 * (c + 1), :],
                            lhsT=u_sb[:, j, :],
                            rhs=xt_sb[:, j * PT:(j + 1) * PT],
                            start=(j // NSTRIP == 0),
                            stop=(j // NSTRIP == KB // NSTRIP - 1),
                            tile_position=(0, 32 * c),
                            skip_group_check=True)

                    # interleave the previous tile's V/y chunks between
                    # groups so PE never head-of-line blocks on the delta
                    # bank rotation.
                    emit_expand_chunks(prev, 1)

                hs_sb = keeps.tile([128, PT], BF16, tag="hs")
                nc.scalar.copy(out=hs_sb, in_=h_psum)
                emit_expand_chunks(prev, NC_CH)  # flush any remainder
                prev = {"hs_sb": hs_sb, "rstd": rstd, "x_sb": x_sb,
                        "t0": t0, "y_sb": None, "n": 0}

            emit_expand_chunks(prev, NC_CH)
    return nc


def _rank_mask_np(S):
    s_abs = np.abs(S)
    cum = np.cumsum(s_abs) / max(float(s_abs.sum()), 1e-8)
    hit = cum >= ENERGY_THRESHOLD
    r = int(np.argmax(hit)) + 1 if hit.any() else S.shape[0]
    return (np.arange(S.shape[0]) < r).astype(S.dtype)


def make_in_maps(x, U, S, V, norm_weight, gamma):
    S = np.asarray(S, dtype=np.float32)
    keep = _rank_mask_np(S)
    U2 = (np.asarray(norm_weight, dtype=np.float32)[:, None]
          * np.asarray(U, dtype=np.float32)
          * (S * keep)[None, :]).astype(NP_BF16)
    U2p = np.zeros((D, RP), dtype=NP_BF16)
    U2p[:, :R] = U2
    U2p = np.ascontiguousarray(
        U2p.reshape(KB, 128, RP).transpose(1, 0, 2).reshape(128, KB * RP))
    V2 = (np.asarray(V, dtype=np.float32)
          * np.asarray(gamma, dtype=np.float32)[None, :]).astype(NP_BF16)
    V2r = np.zeros((128, D), dtype=NP_BF16)
    for c in range(NSTRIP):
        V2r[32 * c:32 * c + R, :] = V2
    eye = np.eye(PT, dtype=NP_BF16)
    xf = np.ascontiguousarray(
        np.asarray(x, dtype=np.float32).reshape(TOK, D)).astype(NP_BF16)
    shards = np.split(xf, N_CORES, axis=0)
    return [{"x": s, "u": U2p, "v": V2r, "eye": eye} for s in shards]


_CACHED_NC = None


def run(x, U, S, V, norm_weight, gamma, trace=False, **kw):
    global _CACHED_NC
    if _CACHED_NC is None:
        _CACHED_NC = build_nc()
    in_maps = make_in_maps(x, U, S, V, norm_weight, gamma)
    res = run_bass_kernel_spmd(_CACHED_NC, in_maps,
                               core_ids=list(range(N_CORES)), trace=trace,
                               **kw)
    outs = [np.asarray(res.results[i]["out"]) for i in range(N_CORES)]
    y = np.concatenate(outs, axis=0).reshape(B, T, D).astype(np.float32)
    return y, res


def kernel(x, U, S, V, norm_weight, gamma):
    y, _ = run(x, U, S, V, norm_weight, gamma, trace=False)
    return y
